# revision 1
# baseline (speedup 1.0000x reference)
"""Trainium2 Bass kernel for nn_DeformableTransformerEncoderLayer (B4,LEN5440,D256,H8,L4,P4).

Self-contained: kernel(**inputs) takes FULL inputs as produced by
setup_inputs(), shards over 8 NeuronCores (core c -> batch c//2, query half
c%2), runs one SPMD Bass program, returns the FULL [4, 5440, 256] output.

v2 layout (per core, Q=2720 queries):
  - value^T channel-permuted halves a=0/1, packed per level into y-pair bf16
    "words" (A: even-y bands, B: odd-y bands) so one GPSIMD ap_gather word is
    (v[y,x], v[y+1,x]) for 128 channels; the x+1 side gathers from a 1-word
    shifted view of the same buffer with the SAME index tile.
  - one fused loop over 8 chunks of 340 queries: offset/attn logits via PE
    matmuls in slot layout [128=(h,l,p), q]; softmax via ones matmuls;
    floor/clip/bilinear border weights via magic-rounding + hat functions
    relu(1-|t +- 0.5|) on the Activation engine; word indices as i16.
  - per chunk: 4 big gathers (2 halves x 2 sides, 5440 idxs each); per
    85-query sub-chunk: slot->channel weight broadcast via indicator matmuls
    (PSUM) + ACT evac, bf16 DVE multiply + halving-tree reduce, fused W_out
    matmul, residual + LN1 via ones-matmul partition sums.
  - FFN + LN2 phase at 340-query chunks, bf16 output (host casts to f32).
"""

import numpy as np
from contextlib import ExitStack

import concourse.bass as bass
import concourse.bacc as bacc
import concourse.tile as tile
import concourse.mybir as mybir
from concourse.bass_utils import run_bass_kernel_spmd

FP32 = mybir.dt.float32
BF16 = mybir.dt.bfloat16
I16 = mybir.dt.int16
AL = mybir.AluOpType
AF = mybir.ActivationFunctionType

B, D, H, L, P, DFF, DH = 4, 256, 8, 4, 4, 1024, 32
SHAPES = ((64, 64), (32, 32), (16, 16), (8, 8))
LEN = 5440
Q = LEN // 2
MC1 = 512           # P1 value matmul chunk
GC = 272            # fused P2+gather+combine chunk (10 chunks, no tail)
SC = 68             # combine / WR / Wout sub-chunk (4 per GC)
QF = 272            # FFN chunk
MAGIC = float(3 << 22)  # 1.5*2^23: x+MAGIC stays in [2^23, 2^24) for |x| < 2^22

TOK_START = [0, 4096, 5120, 5376]
WA = [(h // 2) * w for h, w in SHAPES]
WB = [(h // 2 - 1) * w for h, w in SHAPES]
LBASE = np.concatenate([[0], np.cumsum([a + b for a, b in zip(WA, WB)])[:-1]]).astype(np.int64)
NW = int(sum(WA) + sum(WB))  # 5320

SLOT_L = np.array([(s % 16) // 4 for s in range(128)])
SLOT_H = np.array([s // 16 for s in range(128)])
PERM_A = np.array([(j // 16) * 32 + (j % 16) for j in range(128)])
PERM_B = PERM_A + 16

SC_W2, SC_H2, SC_WA, SC_W, SC_LB, SC_P5, SC_M5 = 0, 1, 2, 3, 4, 5, 6


def _tl(pool, shape, dtype, tag, bufs=None):
    return pool.tile(list(shape), dtype, name=tag, tag=tag, bufs=bufs)


def build_program(debug_taps=False):
    nc = bacc.Bacc("TRN2", target_bir_lowering=False, debug=False, num_devices=8)

    def inp(name, shape, dt=FP32):
        return nc.dram_tensor(name, list(shape), dt, kind="ExternalInput").ap()

    srcT = inp("srcT", (2, 128, LEN), BF16)
    srcqT = inp("srcqT", (2, 128, Q))
    posT = inp("posT", (2, 128, Q))
    refs8 = inp("refs8", (8, Q))
    wv = inp("wv", (2, 2, 128, 128), BF16)
    woff = inp("woff", (2, 2, 128, 128), BF16)
    wattn = inp("wattn", (2, 128, 128), BF16)
    wout = inp("wout", (2, 128, 256), BF16)
    w1 = inp("w1", (2, 128, 1024), BF16)
    w2 = inp("w2", (8, 128, 256), BF16)
    bval = inp("bval", (2, 128, 1))
    bout_ = inp("bout", (2, 128, 1))
    b1_ = inp("b1", (8, 128, 1))
    b2_ = inp("b2", (2, 128, 1))
    ln1g = inp("ln1g", (2, 128, 1))
    ln1b = inp("ln1b", (2, 128, 1))
    ln2g = inp("ln2g", (2, 128, 1))
    ln2b = inp("ln2b", (2, 128, 1))
    boffx = inp("boffx", (128, 1))
    boffy = inp("boffy", (128, 1))
    battn = inp("battn", (128, 1))
    sconst = inp("sconst", (128, 8))
    rscale = inp("rscale", (8, 1))
    rxi = inp("rx", (8, 128))
    ryi = inp("ry", (8, 128))
    sden = inp("sden", (128, 8))
    sbcast = inp("sbcast", (8, 128))
    slp = inp("slp", (16, 128, 128), BF16)
    ones_kb = inp("ones_kb", (128, 1), BF16)
    ones_bb = inp("ones_bb", (1, 128), BF16)

    outT = nc.dram_tensor("outT", [2, 128, Q], BF16, kind="ExternalOutput").ap()
    taps = {}
    if debug_taps:
        def tap(nm, shp, dt=FP32):
            taps[nm] = nc.dram_tensor("tap_" + nm, list(shp), dt,
                                      kind="ExternalOutput").ap()
        tap("PX", (128, Q)); tap("PY", (128, Q))
        tap("widx", (128, Q), I16)
        tap("wt4", (128, Q, 4), BF16)
        tap("vpk", (2, 128, NW))
        tap("xd", (2, 128, Q))
        tap("t5", (2, 128, Q, 2), BF16)
        tap("wr0", (128, SC, 16, 4), BF16)
        tap("gl0", (128, SC * 16))

    with tile.TileContext(nc) as tc, ExitStack() as ctx:
        cp = ctx.enter_context(tc.tile_pool(name="consts", bufs=1))
        live = ctx.enter_context(tc.tile_pool(name="live", bufs=1))

        def ld(pool, ap, tag):
            t = _tl(pool, list(ap.shape), ap.dtype, tag)
            nc.sync.dma_start(out=t[:], in_=ap)
            return t

        c_wv = [[ld(cp, wv[a, k], f"wv{a}{k}") for k in range(2)] for a in range(2)]
        c_wo = [[ld(cp, woff[x, k], f"wo{x}{k}") for k in range(2)] for x in range(2)]
        c_wat = [ld(cp, wattn[k], f"wat{k}") for k in range(2)]
        c_wout = [ld(cp, wout[a], f"wou{a}") for a in range(2)]
        c_bval = [ld(cp, bval[a], f"bv{a}") for a in range(2)]
        c_bout = [ld(cp, bout_[d_], f"bo{d_}") for d_ in range(2)]
        c_b1 = [ld(cp, b1_[n], f"b1{n}") for n in range(8)]
        c_b2 = [ld(cp, b2_[d_], f"b2{d_}") for d_ in range(2)]
        c_l1g = [ld(cp, ln1g[d_], f"l1g{d_}") for d_ in range(2)]
        c_l1b = [ld(cp, ln1b[d_], f"l1b{d_}") for d_ in range(2)]
        c_l2g = [ld(cp, ln2g[d_], f"l2g{d_}") for d_ in range(2)]
        c_l2b = [ld(cp, ln2b[d_], f"l2b{d_}") for d_ in range(2)]
        c_bx = ld(cp, boffx, "bx")
        c_by = ld(cp, boffy, "by")
        c_ba = ld(cp, battn, "ba")
        c_sc = ld(cp, sconst, "sc")
        c_rs = ld(cp, rscale, "rs")
        c_rx = ld(cp, rxi, "rx")
        c_ry = ld(cp, ryi, "ry")
        c_sd = ld(cp, sden, "sd")
        c_sb = ld(cp, sbcast, "sb")
        c_slp = [ld(cp, slp[i], f"slp{i}") for i in range(16)]
        c_okb = ld(cp, ones_kb, "okb")
        c_obb = ld(cp, ones_bb, "obb")

        def sc(i):
            return c_sc[:, i:i + 1]

        val_pk = [_tl(live, [128, NW], FP32, f"vpk{a}") for a in range(2)]
        xT = [_tl(live, [128, Q], BF16, f"xT{d_}") for d_ in range(2)]

        # ================= P1: value + pack ================================
        with tc.tile_pool(name="ph1", bufs=1) as ph1, \
             tc.tile_pool(name="mv1", bufs=3) as mv1, \
             tc.tile_pool(name="pp1", bufs=3, space="PSUM") as pp1:
            v_f32 = [_tl(ph1, [128, LEN], FP32, f"vf{a}") for a in range(2)]
            for m0 in range(0, LEN, MC1):
                mc = min(MC1, LEN - m0)
                s_src = [_tl(mv1, [128, MC1], BF16, f"ms{k}") for k in range(2)]
                for k in range(2):
                    nc.sync.dma_start(out=s_src[k][:, :mc], in_=srcT[k, :, m0:m0 + mc])
                for a in range(2):
                    ps = _tl(pp1, [128, MC1], FP32, "vps")
                    for k in range(2):
                        nc.tensor.matmul(out=ps[:, :mc], lhsT=c_wv[a][k][:],
                                         rhs=s_src[k][:, :mc],
                                         start=(k == 0), stop=(k == 1))
                    nc.scalar.activation(out=v_f32[a][:, m0:m0 + mc], in_=ps[:, :mc],
                                         func=AF.Identity, bias=c_bval[a][:])
            for a in range(2):
                pk_bf = val_pk[a][:].bitcast(BF16)
                for l, (Hl_, Wl_) in enumerate(SHAPES):
                    g3 = v_f32[a][:, TOK_START[l]:TOK_START[l] + Hl_ * Wl_] \
                        .rearrange("p (y x) -> p y x", y=Hl_)
                    a0 = int(LBASE[l]) * 2
                    dstA = pk_bf[:, a0:a0 + WA[l] * 2].rearrange(
                        "p (k x t) -> p k x t", k=Hl_ // 2, x=Wl_)
                    srcA = g3.rearrange("p (k t) x -> p k x t", t=2)
                    nc.vector.tensor_copy(out=dstA, in_=srcA)
                    nb = Hl_ // 2 - 1
                    if nb > 0:
                        b0 = (int(LBASE[l]) + WA[l]) * 2
                        dstB = pk_bf[:, b0:b0 + nb * Wl_ * 2].rearrange(
                            "p (k x t) -> p k x t", k=nb, x=Wl_)
                        srcB = g3[:, 1:1 + 2 * nb, :].rearrange(
                            "p (k t) x -> p k x t", t=2)
                        nc.vector.tensor_copy(out=dstB, in_=srcB)

        # ================= P2+P4 fused: per 340-query chunk ================
        vpk3 = [val_pk[a][:].rearrange("p (n d) -> p n d", d=1) for a in range(2)]
        vpk3s = [val_pk[a][:, 1:].rearrange("p (n d) -> p n d", d=1) for a in range(2)]
        with tc.tile_pool(name="sk", bufs=1) as sk, \
             tc.tile_pool(name="io2", bufs=2) as io2, \
             tc.tile_pool(name="gp", bufs=1) as gp, \
             tc.tile_pool(name="wrp", bufs=2) as wrp, \
             tc.tile_pool(name="lp", bufs=2) as lp, \
             tc.tile_pool(name="pp2", bufs=1, space="PSUM") as pp2, \
             tc.tile_pool(name="pwr", bufs=1, space="PSUM") as pwr, \
             tc.tile_pool(name="pwo", bufs=1, space="PSUM") as pwo, \
             tc.tile_pool(name="pln", bufs=1, space="PSUM") as pln:
            for m in range(Q // GC):
                gsl = slice(m * GC, (m + 1) * GC)
                # ---- queries for this chunk -------------------------------
                qT = []
                for k in range(2):
                    s_sq = _tl(io2, [128, GC], FP32, f"msq{k}", bufs=1)
                    s_po = _tl(io2, [128, GC], FP32, f"mpo{k}", bufs=1)
                    nc.sync.dma_start(out=s_sq[:], in_=srcqT[k, :, gsl])
                    nc.sync.dma_start(out=s_po[:], in_=posT[k, :, gsl])
                    qk = _tl(io2, [128, GC], BF16, f"qT{k}")
                    nc.vector.tensor_tensor(out=qk[:], in0=s_sq[:], in1=s_po[:],
                                            op=AL.add)
                    qT.append(qk)
                s_r8 = _tl(io2, [8, GC], FP32, "r8")
                nc.sync.dma_start(out=s_r8[:], in_=refs8[:, gsl])
                r8c = _tl(sk, [8, GC], FP32, "r8c")
                nc.scalar.activation(out=r8c[:], in_=s_r8[:], func=AF.Copy,
                                     scale=c_rs[:])

                def t_(tag):
                    return _tl(sk, [128, GC], FP32, tag)

                # ---- logits: PX' = px-0.5, PY' = py-0.5, EA = exp(attn) ---
                PX, PY, EA, WA_ = t_("PX"), t_("PY"), t_("EA"), t_("wa")
                for (dst, cw, ind, bia) in ((PX, c_wo[0], c_rx, c_bx),
                                            (PY, c_wo[1], c_ry, c_by)):
                    ps = _tl(pp2, [128, GC], FP32, "pps")
                    nc.tensor.matmul(out=ps[:], lhsT=ind[:], rhs=r8c[:],
                                     start=True, stop=False)
                    for k in range(2):
                        nc.tensor.matmul(out=ps[:], lhsT=cw[k][:], rhs=qT[k][:],
                                         start=False, stop=(k == 1))
                    nc.scalar.activation(out=dst[:], in_=ps[:],
                                         func=AF.Identity, bias=bia[:])
                if debug_taps:
                    nc.sync.dma_start(out=taps["PX"][:, gsl], in_=PX[:])
                    nc.sync.dma_start(out=taps["PY"][:, gsl], in_=PY[:])
                ps = _tl(pp2, [128, GC], FP32, "pps")
                for k in range(2):
                    nc.tensor.matmul(out=ps[:], lhsT=c_wat[k][:], rhs=qT[k][:],
                                     start=(k == 0), stop=(k == 1))
                nc.scalar.activation(out=EA[:], in_=ps[:], func=AF.Exp,
                                     bias=c_ba[:])
                psd = _tl(pp2, [128, GC], FP32, "pps")
                nc.tensor.matmul(out=psd[0:8, :], lhsT=c_sd[:], rhs=EA[:],
                                 start=True, stop=True)
                rec = _tl(sk, [8, GC], FP32, "r8c")
                nc.vector.reciprocal(out=rec[:], in_=psd[0:8, :])
                psb = _tl(pp2, [128, GC], FP32, "pps")
                nc.tensor.matmul(out=psb[:], lhsT=c_sb[:], rhs=rec[:],
                                 start=True, stop=True)
                nc.vector.tensor_tensor(out=WA_[:], in0=EA[:], in1=psb[:],
                                        op=AL.mult)

                # ---- axis math: floor via magic round, hat weights --------
                def axis(PA, hi_idx, k1, k2, wl, wr):
                    # PA holds p' = p-0.5; returns s0 (=k1), weights in wl/wr
                    nc.scalar.activation(out=k1[:], in_=PA[:], func=AF.Copy,
                                         bias=MAGIC)
                    nc.scalar.activation(out=k1[:], in_=k1[:], func=AF.Copy,
                                         bias=-MAGIC)
                    nc.vector.tensor_scalar(out=k1[:], in0=k1[:], scalar1=0.0,
                                            scalar2=sc(hi_idx), op0=AL.max,
                                            op1=AL.min)
                    nc.vector.tensor_tensor(out=k2[:], in0=PA[:], in1=k1[:],
                                            op=AL.subtract)
                    nc.scalar.activation(out=wl[:], in_=k2[:], func=AF.Abs,
                                         bias=sc(SC_P5))
                    nc.scalar.activation(out=wl[:], in_=wl[:], func=AF.Relu,
                                         scale=-1.0, bias=1.0)
                    nc.scalar.activation(out=wr[:], in_=k2[:], func=AF.Abs,
                                         bias=sc(SC_M5))
                    nc.scalar.activation(out=wr[:], in_=wr[:], func=AF.Relu,
                                         scale=-1.0, bias=1.0)

                XS, TX, WXL, WXR = t_("XS"), t_("TX"), t_("WXL"), t_("WXR")
                YS, TY = t_("YS"), t_("TY")
                axis(PX, SC_W2, XS, TX, WXL, WXR)
                WYT, WYB = PX, PY  # PX/PY scratch dead once TX/TY exist
                axis(PY, SC_H2, YS, TY, WYT, WYB)
                nc.vector.tensor_tensor(out=WXL[:], in0=WXL[:], in1=WA_[:],
                                        op=AL.mult)
                nc.vector.tensor_tensor(out=WXR[:], in0=WXR[:], in1=WA_[:],
                                        op=AL.mult)

                # ---- word index: LB + yp*WA + yb*W + xs -------------------
                yb, wf = TX, TY  # reuse scratch (TX/TY dead)
                nc.scalar.activation(out=yb[:], in_=YS[:], func=AF.Copy,
                                     scale=0.5, bias=-0.25)
                nc.scalar.activation(out=yb[:], in_=yb[:], func=AF.Copy,
                                     bias=MAGIC)
                nc.scalar.activation(out=yb[:], in_=yb[:], func=AF.Copy,
                                     bias=-MAGIC)
                nc.vector.scalar_tensor_tensor(out=wf[:], in0=yb[:], scalar=-2.0,
                                               in1=YS[:], op0=AL.mult, op1=AL.add)
                nc.vector.scalar_tensor_tensor(out=wf[:], in0=wf[:],
                                               scalar=sc(SC_WA), in1=XS[:],
                                               op0=AL.mult, op1=AL.add)
                nc.vector.scalar_tensor_tensor(out=wf[:], in0=yb[:],
                                               scalar=sc(SC_W), in1=wf[:],
                                               op0=AL.mult, op1=AL.add)
                widx = _tl(io2, [128, GC], I16, "wi")
                nc.scalar.activation(out=widx[:], in_=wf[:], func=AF.Identity,
                                     bias=sc(SC_LB))
                if debug_taps:
                    nc.sync.dma_start(out=taps["widx"][:, gsl], in_=widx[:])

                # ---- corner weights (bf16, q-major) -----------------------
                wt4 = _tl(io2, [128, GC, 4], BF16, "wt4")
                nc.vector.tensor_tensor(out=wt4[:, :, 0], in0=WXL[:], in1=WYT[:],
                                        op=AL.mult)
                nc.vector.tensor_tensor(out=wt4[:, :, 1], in0=WXL[:], in1=WYB[:],
                                        op=AL.mult)
                nc.vector.tensor_tensor(out=wt4[:, :, 2], in0=WXR[:], in1=WYT[:],
                                        op=AL.mult)
                nc.vector.tensor_tensor(out=wt4[:, :, 3], in0=WXR[:], in1=WYB[:],
                                        op=AL.mult)

                if debug_taps:
                    nc.sync.dma_start(out=taps["wt4"][:, gsl, :], in_=wt4[:])
                    if m == 0:
                        for a_ in range(2):
                            nc.sync.dma_start(out=taps["vpk"][a_],
                                              in_=val_pk[a_][:])
                # ---- gathers: 2 halves x (left, right=shifted src) --------
                G = []
                for a in range(2):
                    GL = _tl(gp, [128, GC * 16], FP32, f"GL{a}",
                             bufs=2 if a == 0 else 1)
                    GR = _tl(gp, [128, GC * 16], FP32, f"GR{a}")
                    nc.gpsimd.ap_gather(
                        out_ap=GL[:].rearrange("p (n d) -> p n d", d=1),
                        in_ap=vpk3[a], idxs_ap=widx[:],
                        channels=128, num_elems=NW, d=1, num_idxs=GC * 16)
                    nc.gpsimd.ap_gather(
                        out_ap=GR[:].rearrange("p (n d) -> p n d", d=1),
                        in_ap=vpk3s[a], idxs_ap=widx[:],
                        channels=128, num_elems=NW - 1, d=1, num_idxs=GC * 16)
                    G.append((GL, GR))

                # ---- per sub-chunk: WR broadcast, combine, Wout -----------
                xd = [_tl(lp, [128, GC], FP32, f"xd{d_}") for d_ in range(2)]
                rsd = [_tl(lp, [128, GC], FP32, f"rs{d_}", bufs=1) for d_ in range(2)]
                for d_ in range(2):
                    nc.sync.dma_start(out=rsd[d_][:], in_=srcqT[d_, :, gsl])
                for j in range(GC // SC):
                    q0 = j * SC
                    if debug_taps and m == 0 and j == 0:
                        nc.sync.dma_start(
                            out=taps["gl0"],
                            in_=G[0][0][:, 0:SC * 16])
                    WR = _tl(wrp, [128, SC, 16, 4], BF16, "WR")
                    for g4 in range(4):
                        pswr = _tl(pwr, [128, 4, 512], FP32, "wrps")
                        for i in range(4):
                            nc.tensor.matmul(
                                out=pswr[:, i, :SC * 4],
                                lhsT=c_slp[g4 * 4 + i][:],
                                rhs=wt4[:, q0:q0 + SC, :],
                                start=True, stop=True)
                        src = pswr[:, :, :SC * 4].rearrange(
                            "p l (q n) -> p q l n", n=4)
                        nc.scalar.activation(out=WR[:, :, g4 * 4:g4 * 4 + 4, :],
                                             in_=src, func=AF.Copy)
                    if debug_taps and m == 0 and j == 0:
                        nc.sync.dma_start(out=taps["wr0"], in_=WR[:])
                    fin = []
                    for a in range(2):
                        GL, GR = G[a]
                        gl = GL[:, q0 * 16:(q0 + SC) * 16].bitcast(BF16).rearrange(
                            "p (q l t) -> p q l t", l=16, t=2)
                        gr = GR[:, q0 * 16:(q0 + SC) * 16].bitcast(BF16).rearrange(
                            "p (q l t) -> p q l t", l=16, t=2)
                        nc.vector.tensor_tensor(out=gl, in0=gl, in1=WR[:, :, :, 0:2],
                                                op=AL.mult)
                        nc.vector.tensor_tensor(out=gr, in0=gr, in1=WR[:, :, :, 2:4],
                                                op=AL.mult)
                        nc.vector.tensor_tensor(out=gl, in0=gl, in1=gr, op=AL.add)
                        # halving tree in place inside the GL buffer
                        nc.vector.tensor_tensor(out=gl[:, :, 0:8, :],
                                                in0=gl[:, :, 0:8, :],
                                                in1=gl[:, :, 8:16, :], op=AL.add)
                        nc.vector.tensor_tensor(out=gl[:, :, 0:4, :],
                                                in0=gl[:, :, 0:4, :],
                                                in1=gl[:, :, 4:8, :], op=AL.add)
                        nc.vector.tensor_tensor(out=gl[:, :, 0:2, :],
                                                in0=gl[:, :, 0:2, :],
                                                in1=gl[:, :, 2:4, :], op=AL.add)
                        nc.vector.tensor_tensor(out=gl[:, :, 0, :],
                                                in0=gl[:, :, 0, :],
                                                in1=gl[:, :, 1, :], op=AL.add)
                        fin.append(gl[:, :, 0, :])
                        if debug_taps:
                            nc.sync.dma_start(
                                out=taps["t5"][a][:, m * GC + q0:m * GC + q0 + SC, :],
                                in_=gl[:, :, 0, :])
                    ps2 = _tl(pwo, [128, 2, SC], FP32, "wops")
                    for d_ in range(2):
                        i = 0
                        for a in range(2):
                            for off in range(2):
                                nc.tensor.matmul(
                                    out=ps2[:, d_],
                                    lhsT=c_wout[a][:, d_ * 128:(d_ + 1) * 128],
                                    rhs=fin[a][:, :, off:off + 1],
                                    start=(i == 0), stop=(i == 3))
                                i += 1
                    for d_ in range(2):
                        nc.scalar.activation(out=xd[d_][:, q0:q0 + SC],
                                             in_=ps2[:, d_], func=AF.Identity,
                                             bias=c_bout[d_][:])
                        nc.vector.tensor_tensor(out=xd[d_][:, q0:q0 + SC],
                                                in0=xd[d_][:, q0:q0 + SC],
                                                in1=rsd[d_][:, q0:q0 + SC],
                                                op=AL.add)
                if debug_taps:
                    for d_ in range(2):
                        nc.sync.dma_start(out=taps["xd"][d_][:, gsl],
                                          in_=xd[d_][:])
                _layer_norm(nc, pln, lp, xd, GC, c_okb, c_obb, c_l1g, c_l1b,
                            [xT[0][:, gsl], xT[1][:, gsl]])

        # ================= P5: FFN + LN2 ===================================
        with tc.tile_pool(name="cp5", bufs=1) as cp5, \
             tc.tile_pool(name="fp", bufs=2) as fp, \
             tc.tile_pool(name="lp2", bufs=2) as lp2, \
             tc.tile_pool(name="pfh", bufs=2, space="PSUM") as pfh, \
             tc.tile_pool(name="pff", bufs=1, space="PSUM") as pff, \
             tc.tile_pool(name="pl2", bufs=1, space="PSUM") as pl2:
            c_w1 = [ld(cp5, w1[k], f"w1{k}") for k in range(2)]
            c_w2 = [ld(cp5, w2[n], f"w2{n}") for n in range(8)]
            for f0 in range(0, Q, QF):
                fsl = slice(f0, f0 + QF)
                hbf = []
                for n in range(8):
                    psh = _tl(pfh, [128, QF], FP32, "psh")
                    for k in range(2):
                        nc.tensor.matmul(out=psh[:], lhsT=c_w1[k][:, n * 128:(n + 1) * 128],
                                         rhs=xT[k][:, fsl], start=(k == 0), stop=(k == 1))
                    hb = _tl(fp, [128, QF], BF16, f"hb{n}")
                    nc.scalar.activation(out=hb[:], in_=psh[:], func=AF.Relu,
                                         bias=c_b1[n][:])
                    hbf.append(hb)
                psf = _tl(pff, [128, 2, 512], FP32, "ffps")
                for d_ in range(2):
                    for n in range(8):
                        nc.tensor.matmul(out=psf[:, d_, :QF],
                                         lhsT=c_w2[n][:, d_ * 128:(d_ + 1) * 128],
                                         rhs=hbf[n][:], start=(n == 0), stop=(n == 7))
                xf = []
                for d_ in range(2):
                    xd2 = _tl(lp2, [128, QF], FP32, f"fx{d_}")
                    nc.scalar.activation(out=xd2[:], in_=psf[:, d_, :QF],
                                         func=AF.Identity, bias=c_b2[d_][:])
                    nc.vector.tensor_tensor(out=xd2[:], in0=xd2[:], in1=xT[d_][:, fsl],
                                            op=AL.add)
                    xf.append(xd2)
                outs = [_tl(lp2, [128, QF], BF16, f"ot{d_}") for d_ in range(2)]
                _layer_norm(nc, pl2, lp2, xf, QF, c_okb, c_obb, c_l2g, c_l2b,
                            [outs[0][:], outs[1][:]])
                for d_ in range(2):
                    nc.sync.dma_start(out=outT[d_, :, fsl], in_=outs[d_][:])

    nc.compile()
    return nc, taps


def _layer_norm(nc, psum_pool, sb_pool, xf, qc, c_okb, c_obb, gain, bias, outs):
    """xf: two [128, qc] f32 tiles (256 channels total). Writes gain*xhat+bias
    into outs (APs pre-sliced to qc; out dtype = AP dtype)."""
    xb, sq = [], []
    for d_ in range(2):
        t = _tl(sb_pool, [128, qc], BF16, f"lnb{d_}", bufs=1)
        nc.scalar.activation(out=t[:], in_=xf[d_][:, :qc], func=AF.Copy)
        xb.append(t)
        t2 = _tl(sb_pool, [128, qc], BF16, f"lnq{d_}", bufs=1)
        nc.scalar.activation(out=t2[:], in_=xf[d_][:, :qc], func=AF.Square)
        sq.append(t2)
    off = qc if 2 * qc <= 512 else 512
    lnp = _tl(psum_pool, [128, off + qc], FP32, "lnp")
    psm_, pss_ = lnp[0:1, 0:qc], lnp[0:1, off:off + qc]
    for d_ in range(2):
        nc.tensor.matmul(out=psm_, lhsT=c_okb[:], rhs=xb[d_][:],
                         start=(d_ == 0), stop=(d_ == 1))
    for d_ in range(2):
        nc.tensor.matmul(out=pss_, lhsT=c_okb[:], rhs=sq[d_][:],
                         start=(d_ == 0), stop=(d_ == 1))
    m_ = _tl(sb_pool, [1, qc], FP32, "m", bufs=1)
    s_ = _tl(sb_pool, [1, qc], FP32, "s", bufs=1)
    nc.scalar.activation(out=m_[:], in_=psm_, func=AF.Copy, scale=1.0 / 256)
    nc.scalar.activation(out=s_[:], in_=pss_, func=AF.Copy, scale=1.0 / 256)
    v_ = _tl(sb_pool, [1, qc], FP32, "vv", bufs=1)
    nc.scalar.activation(out=v_[:], in_=m_[:], func=AF.Square)
    nc.vector.tensor_tensor(out=v_[:], in0=s_[:], in1=v_[:], op=AL.subtract)
    nc.vector.tensor_scalar(out=v_[:], in0=v_[:], scalar1=1e-5,
                            scalar2=None, op0=AL.add)
    r_ = _tl(sb_pool, [1, qc], FP32, "rr", bufs=1)
    nc.scalar.activation(out=r_[:], in_=v_[:], func=AF.Sqrt)
    nc.vector.reciprocal(out=r_[:], in_=r_[:])
    mb = _tl(sb_pool, [1, qc], BF16, "mb", bufs=1)
    rb = _tl(sb_pool, [1, qc], BF16, "rb", bufs=1)
    nc.scalar.activation(out=mb[:], in_=m_[:], func=AF.Copy)
    nc.scalar.activation(out=rb[:], in_=r_[:], func=AF.Copy)
    psM, psR = lnp[:, 0:qc], lnp[:, off:off + qc]
    nc.tensor.matmul(out=psM, lhsT=c_obb[:], rhs=mb[:], start=True, stop=True)
    nc.tensor.matmul(out=psR, lhsT=c_obb[:], rhs=rb[:], start=True, stop=True)
    for d_ in range(2):
        t = _tl(sb_pool, [128, qc], FP32, f"lnt{d_}", bufs=1)
        nc.vector.tensor_tensor(out=t[:], in0=xf[d_][:, :qc], in1=psM,
                                op=AL.subtract)
        nc.vector.tensor_tensor(out=t[:], in0=t[:], in1=psR, op=AL.mult)
        nc.vector.tensor_scalar(out=outs[d_], in0=t[:], scalar1=gain[d_][:],
                                scalar2=bias[d_][:], op0=AL.mult, op1=AL.add)


# --------------------------------------------------------------------------
# host side
# --------------------------------------------------------------------------

def host_consts(inputs):
    import ml_dtypes
    bf = ml_dtypes.bfloat16
    f32 = np.float32
    Wv = np.asarray(inputs["W_value"], f32)
    Woff = np.asarray(inputs["W_off"], f32).reshape(D, H, L, P, 2)
    boff = np.asarray(inputs["b_off"], f32).reshape(H, L, P, 2)
    Wat = np.asarray(inputs["W_attn"], f32).reshape(D, H, L, P)
    bat = np.asarray(inputs["b_attn"], f32).reshape(H, L, P)
    Wout = np.asarray(inputs["W_out"], f32)
    W1 = np.asarray(inputs["W1"], f32)
    W2 = np.asarray(inputs["W2"], f32)
    perm = [PERM_A, PERM_B]
    m = {}
    m["wv"] = np.stack([np.stack([np.ascontiguousarray(Wv[k * 128:(k + 1) * 128][:, perm[a]])
                                  for k in range(2)]) for a in range(2)]).astype(bf)
    wox = Woff[..., 0].reshape(D, 128)
    woy = Woff[..., 1].reshape(D, 128)
    m["woff"] = np.stack([np.stack([wox[k * 128:(k + 1) * 128] for k in range(2)]),
                          np.stack([woy[k * 128:(k + 1) * 128] for k in range(2)])]).astype(bf)
    m["wattn"] = np.stack([Wat.reshape(D, 128)[k * 128:(k + 1) * 128] for k in range(2)]).astype(bf)
    m["wout"] = np.stack([Wout[perm[a], :] for a in range(2)]).astype(bf)
    m["w1"] = np.stack([W1[k * 128:(k + 1) * 128] for k in range(2)]).astype(bf)
    m["w2"] = np.stack([W2[n * 128:(n + 1) * 128] for n in range(8)]).astype(bf)
    bv = np.asarray(inputs["b_value"], f32)
    m["bval"] = np.stack([bv[perm[a]][:, None] for a in range(2)])
    m["bout"] = np.asarray(inputs["b_out"], f32).reshape(2, 128, 1)
    m["b1"] = np.asarray(inputs["b1"], f32).reshape(8, 128, 1)
    m["b2"] = np.asarray(inputs["b2"], f32).reshape(2, 128, 1)
    m["ln1g"] = np.asarray(inputs["ln1_g"], f32).reshape(2, 128, 1)
    m["ln1b"] = np.asarray(inputs["ln1_b"], f32).reshape(2, 128, 1)
    m["ln2g"] = np.asarray(inputs["ln2_g"], f32).reshape(2, 128, 1)
    m["ln2b"] = np.asarray(inputs["ln2_b"], f32).reshape(2, 128, 1)
    # px' = px - 0.5: fold an extra -0.5 (grid-sample) and -0.5 (floor trick)
    m["boffx"] = (boff[..., 0].reshape(128) - 1.0)[:, None].astype(f32)
    m["boffy"] = (boff[..., 1].reshape(128) - 1.0)[:, None].astype(f32)
    m["battn"] = bat.reshape(128)[:, None].astype(f32)
    Wl = np.array([SHAPES[l][1] for l in SLOT_L], f32)
    Hl = np.array([SHAPES[l][0] for l in SLOT_L], f32)
    scn = np.zeros((128, 8), f32)
    scn[:, SC_W2] = Wl - 2.0
    scn[:, SC_H2] = Hl - 2.0
    scn[:, SC_WA] = [WA[l] for l in SLOT_L]
    scn[:, SC_W] = Wl
    scn[:, SC_LB] = LBASE[SLOT_L]
    scn[:, SC_P5] = 0.5
    scn[:, SC_M5] = -0.5
    m["sconst"] = scn
    m["rscale"] = np.array([SHAPES[l][1] for l in range(4)] +
                           [SHAPES[l][0] for l in range(4)], f32)[:, None]
    rx = np.zeros((8, 128), f32)
    ry = np.zeros((8, 128), f32)
    for s in range(128):
        rx[SLOT_L[s], s] = 1.0
        ry[4 + SLOT_L[s], s] = 1.0
    m["rx"], m["ry"] = rx, ry
    sb_ = np.zeros((128, 8), f32)
    for s in range(128):
        sb_[s, SLOT_H[s]] = 1.0
    m["sden"] = sb_
    m["sbcast"] = np.ascontiguousarray(sb_.T)
    slp_ = np.zeros((16, 128, 128), f32)
    for lpi in range(16):
        for h in range(8):
            slp_[lpi, 16 * h + lpi, 16 * h:16 * h + 16] = 1.0
    m["slp"] = slp_.astype(bf)
    m["ones_kb"] = np.ones((128, 1), bf)
    m["ones_bb"] = np.ones((1, 128), bf)
    return m


def host_core_inputs(inputs, core):
    b, half = core // 2, core % 2
    f32 = np.float32
    src = np.asarray(inputs["src"][b], f32)
    pos = np.asarray(inputs["pos"][b], f32)
    refp = np.asarray(inputs["reference_points"][b], f32)
    q0 = half * Q
    import ml_dtypes
    srcT = np.ascontiguousarray(src.T).reshape(2, 128, LEN).astype(ml_dtypes.bfloat16)
    srcqT = np.ascontiguousarray(src[q0:q0 + Q].T).reshape(2, 128, Q)
    posT = np.ascontiguousarray(pos[q0:q0 + Q].T).reshape(2, 128, Q)
    r8 = np.concatenate([refp[q0:q0 + Q, :, 0].T, refp[q0:q0 + Q, :, 1].T], 0)
    return {"srcT": srcT, "srcqT": srcqT, "posT": posT,
            "refs8": np.ascontiguousarray(r8.astype(f32))}


_CACHE = {}


def _run(inputs, trace=False):
    if "nc" not in _CACHE:
        _CACHE["nc"], _ = build_program(debug_taps=False)
    nc = _CACHE["nc"]
    shared = host_consts(inputs)
    in_maps = []
    for c in range(8):
        im = dict(shared)
        im.update(host_core_inputs(inputs, c))
        in_maps.append(im)
    res = run_bass_kernel_spmd(nc, in_maps, list(range(8)), trace=trace)
    out = np.zeros((B, LEN, D), np.float32)
    for c in range(8):
        b, half = c // 2, c % 2
        o = np.asarray(res.results[c]["outT"]).astype(np.float32).reshape(256, Q)
        out[b, half * Q:(half + 1) * Q, :] = o.T
    return out, res


def kernel(**inputs):
    return _run(inputs, trace=False)[0]



# revision 5
# speedup vs baseline: 1.1308x; 1.1308x over previous
"""Trainium2 Bass kernel for nn_DeformableTransformerEncoderLayer (B4,LEN5440,D256,H8,L4,P4).

Self-contained: kernel(**inputs) takes FULL inputs as produced by
setup_inputs(), shards over 8 NeuronCores (core c -> batch c//2, query half
c%2), runs one SPMD Bass program, returns the FULL [4, 5440, 256] output.

v2 layout (per core, Q=2720 queries):
  - value^T channel-permuted halves a=0/1, packed per level into y-pair bf16
    "words" (A: even-y bands, B: odd-y bands) so one GPSIMD ap_gather word is
    (v[y,x], v[y+1,x]) for 128 channels; the x+1 side gathers from a 1-word
    shifted view of the same buffer with the SAME index tile.
  - one fused loop over 8 chunks of 340 queries: offset/attn logits via PE
    matmuls in slot layout [128=(h,l,p), q]; softmax via ones matmuls;
    floor/clip/bilinear border weights via magic-rounding + hat functions
    relu(1-|t +- 0.5|) on the Activation engine; word indices as i16.
  - per chunk: 4 big gathers (2 halves x 2 sides, 5440 idxs each); per
    85-query sub-chunk: slot->channel weight broadcast via indicator matmuls
    (PSUM) + ACT evac, bf16 DVE multiply + halving-tree reduce, fused W_out
    matmul, residual + LN1 via ones-matmul partition sums.
  - FFN + LN2 phase at 340-query chunks, bf16 output (host casts to f32).
"""

import numpy as np
from contextlib import ExitStack

import concourse.bass as bass
import concourse.bacc as bacc
import concourse.tile as tile
import concourse.mybir as mybir
from concourse.bass_utils import run_bass_kernel_spmd

FP32 = mybir.dt.float32
BF16 = mybir.dt.bfloat16
I16 = mybir.dt.int16
AL = mybir.AluOpType
AF = mybir.ActivationFunctionType

B, D, H, L, P, DFF, DH = 4, 256, 8, 4, 4, 1024, 32
SHAPES = ((64, 64), (32, 32), (16, 16), (8, 8))
LEN = 5440
Q = LEN // 2
MC1 = 512           # P1 value matmul chunk
GC = 272            # fused P2+gather+combine chunk (10 chunks, no tail)
SC = 68             # combine / WR / Wout sub-chunk (4 per GC)
QF = 272            # FFN chunk
MAGIC = float(3 << 22)  # 1.5*2^23: x+MAGIC stays in [2^23, 2^24) for |x| < 2^22

TOK_START = [0, 4096, 5120, 5376]
WA = [(h // 2) * w for h, w in SHAPES]
WB = [(h // 2 - 1) * w for h, w in SHAPES]
LBASE = np.concatenate([[0], np.cumsum([a + b for a, b in zip(WA, WB)])[:-1]]).astype(np.int64)
NW = int(sum(WA) + sum(WB))  # 5320

SLOT_L = np.array([(s % 16) // 4 for s in range(128)])
SLOT_H = np.array([s // 16 for s in range(128)])
PERM_A = np.array([(j // 16) * 32 + (j % 16) for j in range(128)])
PERM_B = PERM_A + 16

SC_W2, SC_H2, SC_WA, SC_W, SC_LB, SC_P5, SC_M5 = 0, 1, 2, 3, 4, 5, 6


def _tl(pool, shape, dtype, tag, bufs=None):
    return pool.tile(list(shape), dtype, name=tag, tag=tag, bufs=bufs)


def build_program(debug_taps=False):
    nc = bacc.Bacc("TRN2", target_bir_lowering=False, debug=False, num_devices=8)

    def inp(name, shape, dt=FP32):
        return nc.dram_tensor(name, list(shape), dt, kind="ExternalInput").ap()

    srcT = inp("srcT", (2, 128, LEN), BF16)
    srcqT = inp("srcqT", (2, 128, Q))
    posT = inp("posT", (2, 128, Q))
    refs8 = inp("refs8", (8, Q))
    wv = inp("wv", (2, 2, 128, 128), BF16)
    woff = inp("woff", (2, 2, 128, 128), BF16)
    wattn = inp("wattn", (2, 128, 128), BF16)
    wout = inp("wout", (2, 128, 256), BF16)
    w1 = inp("w1", (2, 128, 1024), BF16)
    w2 = inp("w2", (8, 128, 256), BF16)
    bval = inp("bval", (2, 128, 1))
    bout_ = inp("bout", (2, 128, 1))
    b1_ = inp("b1", (8, 128, 1))
    b2_ = inp("b2", (2, 128, 1))
    ln1g = inp("ln1g", (2, 128, 1))
    ln1b = inp("ln1b", (2, 128, 1))
    ln2g = inp("ln2g", (2, 128, 1))
    ln2b = inp("ln2b", (2, 128, 1))
    boffx = inp("boffx", (128, 1))
    boffy = inp("boffy", (128, 1))
    battn = inp("battn", (128, 1))
    sconst = inp("sconst", (128, 8))
    rscale = inp("rscale", (8, 1))
    rxi = inp("rx", (8, 128))
    ryi = inp("ry", (8, 128))
    sden = inp("sden", (128, 8))
    sbcast = inp("sbcast", (8, 128))
    slp = inp("slp", (16, 128, 128), BF16)
    ones_kb = inp("ones_kb", (128, 1), BF16)
    ones_bb = inp("ones_bb", (1, 128), BF16)

    outT = nc.dram_tensor("outT", [2, 128, Q], BF16, kind="ExternalOutput").ap()
    taps = {}
    if debug_taps:
        def tap(nm, shp, dt=FP32):
            taps[nm] = nc.dram_tensor("tap_" + nm, list(shp), dt,
                                      kind="ExternalOutput").ap()
        tap("PX", (128, Q)); tap("PY", (128, Q))
        tap("widx", (128, Q), I16)
        tap("wt4", (128, Q, 4), BF16)
        tap("vpk", (2, 128, NW))
        tap("xd", (2, 128, Q))
        tap("t5", (2, 128, Q, 2), BF16)
        tap("wr0", (128, SC, 16, 4), BF16)
        tap("gl0", (128, SC * 16))

    with tile.TileContext(nc) as tc, ExitStack() as ctx:
        cp = ctx.enter_context(tc.tile_pool(name="consts", bufs=1))
        live = ctx.enter_context(tc.tile_pool(name="live", bufs=1))

        def ld(pool, ap, tag):
            t = _tl(pool, list(ap.shape), ap.dtype, tag)
            nc.sync.dma_start(out=t[:], in_=ap)
            return t

        c_wv = [[ld(cp, wv[a, k], f"wv{a}{k}") for k in range(2)] for a in range(2)]
        c_wo = [[ld(cp, woff[x, k], f"wo{x}{k}") for k in range(2)] for x in range(2)]
        c_wat = [ld(cp, wattn[k], f"wat{k}") for k in range(2)]
        c_wout = [ld(cp, wout[a], f"wou{a}") for a in range(2)]
        c_bval = [ld(cp, bval[a], f"bv{a}") for a in range(2)]
        c_bout = [ld(cp, bout_[d_], f"bo{d_}") for d_ in range(2)]
        c_b1 = [ld(cp, b1_[n], f"b1{n}") for n in range(8)]
        c_b2 = [ld(cp, b2_[d_], f"b2{d_}") for d_ in range(2)]
        c_l1g = [ld(cp, ln1g[d_], f"l1g{d_}") for d_ in range(2)]
        c_l1b = [ld(cp, ln1b[d_], f"l1b{d_}") for d_ in range(2)]
        c_l2g = [ld(cp, ln2g[d_], f"l2g{d_}") for d_ in range(2)]
        c_l2b = [ld(cp, ln2b[d_], f"l2b{d_}") for d_ in range(2)]
        c_bx = ld(cp, boffx, "bx")
        c_by = ld(cp, boffy, "by")
        c_ba = ld(cp, battn, "ba")
        c_sc = ld(cp, sconst, "sc")
        c_rs = ld(cp, rscale, "rs")
        c_rx = ld(cp, rxi, "rx")
        c_ry = ld(cp, ryi, "ry")
        c_sd = ld(cp, sden, "sd")
        c_sb = ld(cp, sbcast, "sb")
        c_slp = [ld(cp, slp[i], f"slp{i}") for i in range(16)]
        c_okb = ld(cp, ones_kb, "okb")
        c_obb = ld(cp, ones_bb, "obb")
        c_w1 = [ld(cp, w1[k], f"w1{k}") for k in range(2)]
        c_w2 = [ld(cp, w2[n], f"w2{n}") for n in range(8)]

        def sc(i):
            return c_sc[:, i:i + 1]

        val_pk = [_tl(live, [128, NW], FP32, f"vpk{a}") for a in range(2)]

        # ================= P1: value + pack ================================
        with tc.tile_pool(name="ph1", bufs=1) as ph1, \
             tc.tile_pool(name="mv1", bufs=3) as mv1, \
             tc.tile_pool(name="pp1", bufs=3, space="PSUM") as pp1:
            v_f32 = [_tl(ph1, [128, LEN], FP32, f"vf{a}") for a in range(2)]
            for m0 in range(0, LEN, MC1):
                mc = min(MC1, LEN - m0)
                s_src = [_tl(mv1, [128, MC1], BF16, f"ms{k}") for k in range(2)]
                for k in range(2):
                    nc.sync.dma_start(out=s_src[k][:, :mc], in_=srcT[k, :, m0:m0 + mc])
                for a in range(2):
                    ps = _tl(pp1, [128, MC1], FP32, "vps")
                    for k in range(2):
                        nc.tensor.matmul(out=ps[:, :mc], lhsT=c_wv[a][k][:],
                                         rhs=s_src[k][:, :mc],
                                         start=(k == 0), stop=(k == 1))
                    nc.scalar.activation(out=v_f32[a][:, m0:m0 + mc], in_=ps[:, :mc],
                                         func=AF.Identity, bias=c_bval[a][:])
            for a in range(2):
                pk_bf = val_pk[a][:].bitcast(BF16)
                for l, (Hl_, Wl_) in enumerate(SHAPES):
                    g3 = v_f32[a][:, TOK_START[l]:TOK_START[l] + Hl_ * Wl_] \
                        .rearrange("p (y x) -> p y x", y=Hl_)
                    a0 = int(LBASE[l]) * 2
                    dstA = pk_bf[:, a0:a0 + WA[l] * 2].rearrange(
                        "p (k x t) -> p k x t", k=Hl_ // 2, x=Wl_)
                    srcA = g3.rearrange("p (k t) x -> p k x t", t=2)
                    nc.vector.tensor_copy(out=dstA, in_=srcA)
                    nb = Hl_ // 2 - 1
                    if nb > 0:
                        b0 = (int(LBASE[l]) + WA[l]) * 2
                        dstB = pk_bf[:, b0:b0 + nb * Wl_ * 2].rearrange(
                            "p (k x t) -> p k x t", k=nb, x=Wl_)
                        srcB = g3[:, 1:1 + 2 * nb, :].rearrange(
                            "p (k t) x -> p k x t", t=2)
                        nc.vector.tensor_copy(out=dstB, in_=srcB)

        # ====== P2+P4+FFN fused: per 272-query chunk, L/R side phases ======
        vpk3 = [val_pk[a][:].rearrange("p (n d) -> p n d", d=1) for a in range(2)]
        vpk3s = [val_pk[a][:, 1:].rearrange("p (n d) -> p n d", d=1) for a in range(2)]
        with tc.tile_pool(name="sk", bufs=1) as sk, \
             tc.tile_pool(name="io2", bufs=2) as io2, \
             tc.tile_pool(name="gp", bufs=1) as gp, \
             tc.tile_pool(name="wrp", bufs=2) as wrp, \
             tc.tile_pool(name="lp", bufs=2) as lp, \
             tc.tile_pool(name="xtp", bufs=2) as xtp, \
             tc.tile_pool(name="fp", bufs=1) as fp, \
             tc.tile_pool(name="pp2", bufs=1, space="PSUM") as pp2, \
             tc.tile_pool(name="pwr", bufs=1, space="PSUM") as pwr, \
             tc.tile_pool(name="pwo", bufs=1, space="PSUM") as pwo, \
             tc.tile_pool(name="pfh", bufs=1, space="PSUM") as pfh, \
             tc.tile_pool(name="pff", bufs=1, space="PSUM") as pff, \
             tc.tile_pool(name="pln", bufs=1, space="PSUM") as pln:
            for m in range(Q // GC):
                gsl = slice(m * GC, (m + 1) * GC)
                # ---- queries for this chunk -------------------------------
                qT = []
                for k in range(2):
                    s_sq = _tl(io2, [128, GC], FP32, f"msq{k}", bufs=1)
                    s_po = _tl(io2, [128, GC], FP32, f"mpo{k}", bufs=1)
                    nc.sync.dma_start(out=s_sq[:], in_=srcqT[k, :, gsl])
                    nc.sync.dma_start(out=s_po[:], in_=posT[k, :, gsl])
                    qk = _tl(io2, [128, GC], BF16, f"qT{k}")
                    nc.vector.tensor_tensor(out=qk[:], in0=s_sq[:], in1=s_po[:],
                                            op=AL.add)
                    qT.append(qk)
                s_r8 = _tl(io2, [8, GC], FP32, "r8")
                nc.sync.dma_start(out=s_r8[:], in_=refs8[:, gsl])
                r8c = _tl(sk, [8, GC], FP32, "r8c")
                nc.scalar.activation(out=r8c[:], in_=s_r8[:], func=AF.Copy,
                                     scale=c_rs[:])

                def t_(tag):
                    return _tl(sk, [128, GC], FP32, tag)

                # ---- logits: PX' = px-0.5, PY' = py-0.5, EA = exp(attn) ---
                PX, PY, EA, WA_ = t_("PX"), t_("PY"), t_("EA"), t_("wa")
                for (dst, cw, ind, bia) in ((PX, c_wo[0], c_rx, c_bx),
                                            (PY, c_wo[1], c_ry, c_by)):
                    ps = _tl(pp2, [128, GC], FP32, "pps")
                    nc.tensor.matmul(out=ps[:], lhsT=ind[:], rhs=r8c[:],
                                     start=True, stop=False)
                    for k in range(2):
                        nc.tensor.matmul(out=ps[:], lhsT=cw[k][:], rhs=qT[k][:],
                                         start=False, stop=(k == 1))
                    nc.scalar.activation(out=dst[:], in_=ps[:],
                                         func=AF.Identity, bias=bia[:])
                if debug_taps:
                    nc.sync.dma_start(out=taps["PX"][:, gsl], in_=PX[:])
                    nc.sync.dma_start(out=taps["PY"][:, gsl], in_=PY[:])
                ps = _tl(pp2, [128, GC], FP32, "pps")
                for k in range(2):
                    nc.tensor.matmul(out=ps[:], lhsT=c_wat[k][:], rhs=qT[k][:],
                                     start=(k == 0), stop=(k == 1))
                nc.scalar.activation(out=EA[:], in_=ps[:], func=AF.Exp,
                                     bias=c_ba[:])
                psd = _tl(pp2, [128, GC], FP32, "pps")
                nc.tensor.matmul(out=psd[0:8, :], lhsT=c_sd[:], rhs=EA[:],
                                 start=True, stop=True)
                rec = _tl(sk, [8, GC], FP32, "r8c")
                nc.vector.reciprocal(out=rec[:], in_=psd[0:8, :])
                psb = _tl(pp2, [128, GC], FP32, "pps")
                nc.tensor.matmul(out=psb[:], lhsT=c_sb[:], rhs=rec[:],
                                 start=True, stop=True)
                nc.vector.tensor_tensor(out=WA_[:], in0=EA[:], in1=psb[:],
                                        op=AL.mult)

                # ---- axis math: floor via magic round, hat weights --------
                def axis(PA, hi_idx, k1, k2, wl, wr):
                    # PA holds p' = p-0.5; returns s0 (=k1), weights in wl/wr
                    nc.scalar.activation(out=k1[:], in_=PA[:], func=AF.Copy,
                                         bias=MAGIC)
                    nc.scalar.activation(out=k1[:], in_=k1[:], func=AF.Copy,
                                         bias=-MAGIC)
                    nc.vector.tensor_scalar(out=k1[:], in0=k1[:], scalar1=0.0,
                                            scalar2=sc(hi_idx), op0=AL.max,
                                            op1=AL.min)
                    nc.vector.tensor_tensor(out=k2[:], in0=PA[:], in1=k1[:],
                                            op=AL.subtract)
                    nc.scalar.activation(out=wl[:], in_=k2[:], func=AF.Abs,
                                         bias=sc(SC_P5))
                    nc.scalar.activation(out=wl[:], in_=wl[:], func=AF.Relu,
                                         scale=-1.0, bias=1.0)
                    nc.scalar.activation(out=wr[:], in_=k2[:], func=AF.Abs,
                                         bias=sc(SC_M5))
                    nc.scalar.activation(out=wr[:], in_=wr[:], func=AF.Relu,
                                         scale=-1.0, bias=1.0)

                XS, TX, WXL, WXR = t_("XS"), t_("TX"), t_("WXL"), t_("WXR")
                YS, TY = t_("YS"), t_("TY")
                axis(PX, SC_W2, XS, TX, WXL, WXR)
                WYT, WYB = PX, PY  # PX/PY scratch dead once TX/TY exist
                axis(PY, SC_H2, YS, TY, WYT, WYB)
                nc.vector.tensor_tensor(out=WXL[:], in0=WXL[:], in1=WA_[:],
                                        op=AL.mult)
                nc.vector.tensor_tensor(out=WXR[:], in0=WXR[:], in1=WA_[:],
                                        op=AL.mult)

                # ---- word index: LB + yp*WA + yb*W + xs -------------------
                yb, wf = TX, TY  # reuse scratch (TX/TY dead)
                nc.scalar.activation(out=yb[:], in_=YS[:], func=AF.Copy,
                                     scale=0.5, bias=-0.25)
                nc.scalar.activation(out=yb[:], in_=yb[:], func=AF.Copy,
                                     bias=MAGIC)
                nc.scalar.activation(out=yb[:], in_=yb[:], func=AF.Copy,
                                     bias=-MAGIC)
                nc.vector.scalar_tensor_tensor(out=wf[:], in0=yb[:], scalar=-2.0,
                                               in1=YS[:], op0=AL.mult, op1=AL.add)
                nc.vector.scalar_tensor_tensor(out=wf[:], in0=wf[:],
                                               scalar=sc(SC_WA), in1=XS[:],
                                               op0=AL.mult, op1=AL.add)
                nc.vector.scalar_tensor_tensor(out=wf[:], in0=yb[:],
                                               scalar=sc(SC_W), in1=wf[:],
                                               op0=AL.mult, op1=AL.add)
                widx = _tl(io2, [128, GC], I16, "wi")
                nc.scalar.activation(out=widx[:], in_=wf[:], func=AF.Identity,
                                     bias=sc(SC_LB))
                if debug_taps:
                    nc.sync.dma_start(out=taps["widx"][:, gsl], in_=widx[:])

                # ---- corner weights (bf16, q-major), split L/R ------------
                wt4L = _tl(io2, [128, GC, 2], BF16, "wt4L")
                wt4R = _tl(io2, [128, GC, 2], BF16, "wt4R")
                nc.vector.tensor_tensor(out=wt4L[:, :, 0], in0=WXL[:], in1=WYT[:],
                                        op=AL.mult)
                nc.vector.tensor_tensor(out=wt4L[:, :, 1], in0=WXL[:], in1=WYB[:],
                                        op=AL.mult)
                nc.vector.tensor_tensor(out=wt4R[:, :, 0], in0=WXR[:], in1=WYT[:],
                                        op=AL.mult)
                nc.vector.tensor_tensor(out=wt4R[:, :, 1], in0=WXR[:], in1=WYB[:],
                                        op=AL.mult)

                # ---- gathers: left pair first, right pair second ----------
                GLs, GRs = [], []
                for a in range(2):
                    GL = _tl(gp, [128, GC * 16], FP32, f"GL{a}")
                    nc.gpsimd.ap_gather(
                        out_ap=GL[:].rearrange("p (n d) -> p n d", d=1),
                        in_ap=vpk3[a], idxs_ap=widx[:],
                        channels=128, num_elems=NW, d=1, num_idxs=GC * 16)
                    GLs.append(GL)
                for a in range(2):
                    GR = _tl(gp, [128, GC * 16], FP32, f"GR{a}")
                    nc.gpsimd.ap_gather(
                        out_ap=GR[:].rearrange("p (n d) -> p n d", d=1),
                        in_ap=vpk3s[a], idxs_ap=widx[:],
                        channels=128, num_elems=NW - 1, d=1, num_idxs=GC * 16)
                    GRs.append(GR)

                def gview(t, q0):
                    return t[:, q0 * 16:(q0 + SC) * 16].bitcast(BF16).rearrange(
                        "p (q l t) -> p q l t", l=16, t=2)

                def wr_bcast(wt2, q0, tag):
                    # broadcast [128slot, SC, 2] -> [128chan, SC, 16, 2]
                    WRh = _tl(wrp, [128, SC, 16, 2], BF16, tag)
                    for g4 in range(4):
                        pswr = _tl(pwr, [128, 4, 256], FP32, "wrps")
                        for i in range(4):
                            nc.tensor.matmul(
                                out=pswr[:, i, :SC * 2],
                                lhsT=c_slp[g4 * 4 + i][:],
                                rhs=wt2[:, q0:q0 + SC, :],
                                start=True, stop=True)
                        src = pswr[:, :, :SC * 2].rearrange(
                            "p l (q n) -> p q l n", n=2)
                        nc.scalar.activation(out=WRh[:, :, g4 * 4:g4 * 4 + 4, :],
                                             in_=src, func=AF.Copy)
                    return WRh

                xd = [_tl(lp, [128, GC], FP32, f"xd{d_}") for d_ in range(2)]
                rsd = [_tl(lp, [128, GC], FP32, f"rs{d_}", bufs=1) for d_ in range(2)]
                for d_ in range(2):
                    nc.sync.dma_start(out=rsd[d_][:], in_=srcqT[d_, :, gsl])

                # ---- phase L: weight the left-side words (in place) -------
                for j in range(GC // SC):
                    q0 = j * SC
                    WRL = wr_bcast(wt4L, q0, "WRL")
                    for a in range(2):
                        gl = gview(GLs[a], q0)
                        nc.vector.tensor_tensor(out=gl, in0=gl, in1=WRL[:],
                                                op=AL.mult)
                # ---- phase R: weight right side, add, reduce, Wout --------
                for j in range(GC // SC):
                    q0 = j * SC
                    WRR = wr_bcast(wt4R, q0, "WRR")
                    fin = []
                    for a in range(2):
                        gl = gview(GLs[a], q0)
                        gr = gview(GRs[a], q0)
                        nc.vector.tensor_tensor(out=gr, in0=gr, in1=WRR[:],
                                                op=AL.mult)
                        nc.vector.tensor_tensor(out=gr, in0=gr, in1=gl, op=AL.add)
                        # halving tree in place inside the GR buffer
                        nc.vector.tensor_tensor(out=gr[:, :, 0:8, :],
                                                in0=gr[:, :, 0:8, :],
                                                in1=gr[:, :, 8:16, :], op=AL.add)
                        nc.vector.tensor_tensor(out=gr[:, :, 0:4, :],
                                                in0=gr[:, :, 0:4, :],
                                                in1=gr[:, :, 4:8, :], op=AL.add)
                        nc.vector.tensor_tensor(out=gr[:, :, 0:2, :],
                                                in0=gr[:, :, 0:2, :],
                                                in1=gr[:, :, 2:4, :], op=AL.add)
                        nc.vector.tensor_tensor(out=gr[:, :, 0, :],
                                                in0=gr[:, :, 0, :],
                                                in1=gr[:, :, 1, :], op=AL.add)
                        fin.append(gr[:, :, 0, :])
                        if debug_taps:
                            nc.sync.dma_start(
                                out=taps["t5"][a][:, m * GC + q0:m * GC + q0 + SC, :],
                                in_=gr[:, :, 0, :])
                    ps2 = _tl(pwo, [128, 2, SC], FP32, "wops")
                    for d_ in range(2):
                        i = 0
                        for a in range(2):
                            for off in range(2):
                                nc.tensor.matmul(
                                    out=ps2[:, d_],
                                    lhsT=c_wout[a][:, d_ * 128:(d_ + 1) * 128],
                                    rhs=fin[a][:, :, off:off + 1],
                                    start=(i == 0), stop=(i == 3))
                                i += 1
                    for d_ in range(2):
                        nc.scalar.activation(out=xd[d_][:, q0:q0 + SC],
                                             in_=ps2[:, d_], func=AF.Identity,
                                             bias=c_bout[d_][:])
                        nc.vector.tensor_tensor(out=xd[d_][:, q0:q0 + SC],
                                                in0=xd[d_][:, q0:q0 + SC],
                                                in1=rsd[d_][:, q0:q0 + SC],
                                                op=AL.add)
                if debug_taps:
                    for d_ in range(2):
                        nc.sync.dma_start(out=taps["xd"][d_][:, gsl],
                                          in_=xd[d_][:])
                xTc = [_tl(xtp, [128, GC], BF16, f"xTc{k}") for k in range(2)]
                _layer_norm(nc, pln, lp, xd, GC, c_okb, c_obb, c_l1g, c_l1b,
                            [xTc[0][:], xTc[1][:]])

                # ---- FFN + LN2 for this chunk (fills engine gaps) ---------
                hbf = []
                for n in range(8):
                    psh = _tl(pfh, [128, GC], FP32, "psh")
                    for k in range(2):
                        nc.tensor.matmul(out=psh[:],
                                         lhsT=c_w1[k][:, n * 128:(n + 1) * 128],
                                         rhs=xTc[k][:], start=(k == 0),
                                         stop=(k == 1))
                    hb = _tl(fp, [128, GC], BF16, f"hb{n}")
                    nc.scalar.activation(out=hb[:], in_=psh[:], func=AF.Relu,
                                         bias=c_b1[n][:])
                    hbf.append(hb)
                xf = []
                for d_ in range(2):
                    psf = _tl(pff, [128, 512], FP32, "ffps")
                    for n in range(8):
                        nc.tensor.matmul(out=psf[:, :GC],
                                         lhsT=c_w2[n][:, d_ * 128:(d_ + 1) * 128],
                                         rhs=hbf[n][:], start=(n == 0),
                                         stop=(n == 7))
                    xd2 = _tl(lp, [128, GC], FP32, f"fx{d_}")
                    nc.scalar.activation(out=xd2[:], in_=psf[:, :GC],
                                         func=AF.Identity, bias=c_b2[d_][:])
                    nc.vector.tensor_tensor(out=xd2[:], in0=xd2[:],
                                            in1=xTc[d_][:], op=AL.add)
                    xf.append(xd2)
                outs = [_tl(lp, [128, GC], BF16, f"ot{d_}") for d_ in range(2)]
                _layer_norm(nc, pln, lp, xf, GC, c_okb, c_obb, c_l2g, c_l2b,
                            [outs[0][:], outs[1][:]])
                for d_ in range(2):
                    nc.sync.dma_start(out=outT[d_, :, gsl], in_=outs[d_][:])

    nc.compile()
    return nc, taps


def _layer_norm(nc, psum_pool, sb_pool, xf, qc, c_okb, c_obb, gain, bias, outs):
    """xf: two [128, qc] f32 tiles (256 channels total). Writes gain*xhat+bias
    into outs (APs pre-sliced to qc; out dtype = AP dtype)."""
    xb, sq = [], []
    for d_ in range(2):
        t = _tl(sb_pool, [128, qc], BF16, f"lnb{d_}", bufs=1)
        nc.scalar.activation(out=t[:], in_=xf[d_][:, :qc], func=AF.Copy)
        xb.append(t)
        t2 = _tl(sb_pool, [128, qc], BF16, f"lnq{d_}", bufs=1)
        nc.scalar.activation(out=t2[:], in_=xf[d_][:, :qc], func=AF.Square)
        sq.append(t2)
    off = qc if 2 * qc <= 512 else 512
    lnp = _tl(psum_pool, [128, off + qc], FP32, "lnp")
    psm_, pss_ = lnp[0:1, 0:qc], lnp[0:1, off:off + qc]
    for d_ in range(2):
        nc.tensor.matmul(out=psm_, lhsT=c_okb[:], rhs=xb[d_][:],
                         start=(d_ == 0), stop=(d_ == 1))
    for d_ in range(2):
        nc.tensor.matmul(out=pss_, lhsT=c_okb[:], rhs=sq[d_][:],
                         start=(d_ == 0), stop=(d_ == 1))
    m_ = _tl(sb_pool, [1, qc], FP32, "m", bufs=1)
    s_ = _tl(sb_pool, [1, qc], FP32, "s", bufs=1)
    nc.scalar.activation(out=m_[:], in_=psm_, func=AF.Copy, scale=1.0 / 256)
    nc.scalar.activation(out=s_[:], in_=pss_, func=AF.Copy, scale=1.0 / 256)
    v_ = _tl(sb_pool, [1, qc], FP32, "vv", bufs=1)
    nc.scalar.activation(out=v_[:], in_=m_[:], func=AF.Square)
    nc.vector.tensor_tensor(out=v_[:], in0=s_[:], in1=v_[:], op=AL.subtract)
    nc.vector.tensor_scalar(out=v_[:], in0=v_[:], scalar1=1e-5,
                            scalar2=None, op0=AL.add)
    r_ = _tl(sb_pool, [1, qc], FP32, "rr", bufs=1)
    nc.scalar.activation(out=r_[:], in_=v_[:], func=AF.Sqrt)
    nc.vector.reciprocal(out=r_[:], in_=r_[:])
    mb = _tl(sb_pool, [1, qc], BF16, "mb", bufs=1)
    rb = _tl(sb_pool, [1, qc], BF16, "rb", bufs=1)
    nc.scalar.activation(out=mb[:], in_=m_[:], func=AF.Copy)
    nc.scalar.activation(out=rb[:], in_=r_[:], func=AF.Copy)
    psM, psR = lnp[:, 0:qc], lnp[:, off:off + qc]
    nc.tensor.matmul(out=psM, lhsT=c_obb[:], rhs=mb[:], start=True, stop=True)
    nc.tensor.matmul(out=psR, lhsT=c_obb[:], rhs=rb[:], start=True, stop=True)
    for d_ in range(2):
        t = _tl(sb_pool, [128, qc], FP32, f"lnt{d_}", bufs=1)
        nc.vector.tensor_tensor(out=t[:], in0=xf[d_][:, :qc], in1=psM,
                                op=AL.subtract)
        nc.vector.tensor_tensor(out=t[:], in0=t[:], in1=psR, op=AL.mult)
        nc.vector.tensor_scalar(out=outs[d_], in0=t[:], scalar1=gain[d_][:],
                                scalar2=bias[d_][:], op0=AL.mult, op1=AL.add)


# --------------------------------------------------------------------------
# host side
# --------------------------------------------------------------------------

def host_consts(inputs):
    import ml_dtypes
    bf = ml_dtypes.bfloat16
    f32 = np.float32
    Wv = np.asarray(inputs["W_value"], f32)
    Woff = np.asarray(inputs["W_off"], f32).reshape(D, H, L, P, 2)
    boff = np.asarray(inputs["b_off"], f32).reshape(H, L, P, 2)
    Wat = np.asarray(inputs["W_attn"], f32).reshape(D, H, L, P)
    bat = np.asarray(inputs["b_attn"], f32).reshape(H, L, P)
    Wout = np.asarray(inputs["W_out"], f32)
    W1 = np.asarray(inputs["W1"], f32)
    W2 = np.asarray(inputs["W2"], f32)
    perm = [PERM_A, PERM_B]
    m = {}
    m["wv"] = np.stack([np.stack([np.ascontiguousarray(Wv[k * 128:(k + 1) * 128][:, perm[a]])
                                  for k in range(2)]) for a in range(2)]).astype(bf)
    wox = Woff[..., 0].reshape(D, 128)
    woy = Woff[..., 1].reshape(D, 128)
    m["woff"] = np.stack([np.stack([wox[k * 128:(k + 1) * 128] for k in range(2)]),
                          np.stack([woy[k * 128:(k + 1) * 128] for k in range(2)])]).astype(bf)
    m["wattn"] = np.stack([Wat.reshape(D, 128)[k * 128:(k + 1) * 128] for k in range(2)]).astype(bf)
    m["wout"] = np.stack([Wout[perm[a], :] for a in range(2)]).astype(bf)
    m["w1"] = np.stack([W1[k * 128:(k + 1) * 128] for k in range(2)]).astype(bf)
    m["w2"] = np.stack([W2[n * 128:(n + 1) * 128] for n in range(8)]).astype(bf)
    bv = np.asarray(inputs["b_value"], f32)
    m["bval"] = np.stack([bv[perm[a]][:, None] for a in range(2)])
    m["bout"] = np.asarray(inputs["b_out"], f32).reshape(2, 128, 1)
    m["b1"] = np.asarray(inputs["b1"], f32).reshape(8, 128, 1)
    m["b2"] = np.asarray(inputs["b2"], f32).reshape(2, 128, 1)
    m["ln1g"] = np.asarray(inputs["ln1_g"], f32).reshape(2, 128, 1)
    m["ln1b"] = np.asarray(inputs["ln1_b"], f32).reshape(2, 128, 1)
    m["ln2g"] = np.asarray(inputs["ln2_g"], f32).reshape(2, 128, 1)
    m["ln2b"] = np.asarray(inputs["ln2_b"], f32).reshape(2, 128, 1)
    # px' = px - 0.5: fold an extra -0.5 (grid-sample) and -0.5 (floor trick)
    m["boffx"] = (boff[..., 0].reshape(128) - 1.0)[:, None].astype(f32)
    m["boffy"] = (boff[..., 1].reshape(128) - 1.0)[:, None].astype(f32)
    m["battn"] = bat.reshape(128)[:, None].astype(f32)
    Wl = np.array([SHAPES[l][1] for l in SLOT_L], f32)
    Hl = np.array([SHAPES[l][0] for l in SLOT_L], f32)
    scn = np.zeros((128, 8), f32)
    scn[:, SC_W2] = Wl - 2.0
    scn[:, SC_H2] = Hl - 2.0
    scn[:, SC_WA] = [WA[l] for l in SLOT_L]
    scn[:, SC_W] = Wl
    scn[:, SC_LB] = LBASE[SLOT_L]
    scn[:, SC_P5] = 0.5
    scn[:, SC_M5] = -0.5
    m["sconst"] = scn
    m["rscale"] = np.array([SHAPES[l][1] for l in range(4)] +
                           [SHAPES[l][0] for l in range(4)], f32)[:, None]
    rx = np.zeros((8, 128), f32)
    ry = np.zeros((8, 128), f32)
    for s in range(128):
        rx[SLOT_L[s], s] = 1.0
        ry[4 + SLOT_L[s], s] = 1.0
    m["rx"], m["ry"] = rx, ry
    sb_ = np.zeros((128, 8), f32)
    for s in range(128):
        sb_[s, SLOT_H[s]] = 1.0
    m["sden"] = sb_
    m["sbcast"] = np.ascontiguousarray(sb_.T)
    slp_ = np.zeros((16, 128, 128), f32)
    for lpi in range(16):
        for h in range(8):
            slp_[lpi, 16 * h + lpi, 16 * h:16 * h + 16] = 1.0
    m["slp"] = slp_.astype(bf)
    m["ones_kb"] = np.ones((128, 1), bf)
    m["ones_bb"] = np.ones((1, 128), bf)
    return m


def host_core_inputs(inputs, core):
    b, half = core // 2, core % 2
    f32 = np.float32
    src = np.asarray(inputs["src"][b], f32)
    pos = np.asarray(inputs["pos"][b], f32)
    refp = np.asarray(inputs["reference_points"][b], f32)
    q0 = half * Q
    import ml_dtypes
    srcT = np.ascontiguousarray(src.T).reshape(2, 128, LEN).astype(ml_dtypes.bfloat16)
    srcqT = np.ascontiguousarray(src[q0:q0 + Q].T).reshape(2, 128, Q)
    posT = np.ascontiguousarray(pos[q0:q0 + Q].T).reshape(2, 128, Q)
    r8 = np.concatenate([refp[q0:q0 + Q, :, 0].T, refp[q0:q0 + Q, :, 1].T], 0)
    return {"srcT": srcT, "srcqT": srcqT, "posT": posT,
            "refs8": np.ascontiguousarray(r8.astype(f32))}


_CACHE = {}


def _run(inputs, trace=False):
    if "nc" not in _CACHE:
        _CACHE["nc"], _ = build_program(debug_taps=False)
    nc = _CACHE["nc"]
    shared = host_consts(inputs)
    in_maps = []
    for c in range(8):
        im = dict(shared)
        im.update(host_core_inputs(inputs, c))
        in_maps.append(im)
    res = run_bass_kernel_spmd(nc, in_maps, list(range(8)), trace=trace)
    out = np.zeros((B, LEN, D), np.float32)
    for c in range(8):
        b, half = c // 2, c % 2
        o = np.asarray(res.results[c]["outT"]).astype(np.float32).reshape(256, Q)
        out[b, half * Q:(half + 1) * Q, :] = o.T
    return out, res


def kernel(**inputs):
    return _run(inputs, trace=False)[0]

